# revision 16
# baseline (speedup 1.0000x reference)
"""Trainium2 Bass kernel for nn_DenseAttention (feature-axis attention over a
huge batch), data-parallel over 8 NeuronCores.

Math restructure (per core, batch shard x_s of 32768 rows):
  scores = q.T @ k contracts over batch -> scores = Wq G Wk.T + rank-1 bias
  terms, with G = x.T x (feature Gram) and s = x.T 1. The reference's flat
  reshape maps attn columns to per-tile output rows, so output collapses per
  128-row tile c to  y_block = Mv @ V_cT + corr,  V_cT = X_c.T @ Wo.T, with
  Mv = softmax_weights @ Wv and corr = (weights@bv) x (Wo@1) + bo.

Schedule: G-phase -> G2 residual -> prescore+AllReduce -> V-phase ->
keepalive -> softmax smalls -> pass 2.
  1. G-phase: stream x_hi (bf16, host-transposed so each DMA line is 4KB
     contiguous; x_hi stays resident, 65KB/partition) and accumulate G|s in
     one PSUM bank. G2 streams the bf16 residual x_lo = x - bf16(x) and
     accumulates C = x_lo.T @ [x_hi|1]; G + C + C.T restores ~fp32-grade
     scores from pure-bf16 matmuls (bf16 x alone injects +-24 into scores
     whose min top-2 gap is ~2 -> softmax row flips).
  2. Scores are linear in (G, s), so the local scores contribution is
     computed pre-AR (with B/NCORES on the bq x bk term) and ONE fp32
     AllReduce of [128,128] runs while the V-phase computes from SBUF.
     The CC-stream init barrier is pinned to ~70-75us after NEFF start
     regardless of trigger time, so the AR cannot finish before ~95us.
  3. bf16 keepalive matmuls fill the PE idle window: the HAM clock gate
     re-throttles to k=4/8 after ~3us of PE idle (fp32r/fp16 do not even
     count as busy; bf16 does) and would otherwise halve pass-2 throughput.
  4. pass 2: y_block = Mv @ V_cT per 4 tiles, fp16 output (the rank-1
     corr term cc x (Wo@1) + bo is added on the host from the tiny cc
     output); rel-err budget is 2e-2, measured ~1e-3.
"""
import functools

import numpy as np

try:
    from ml_dtypes import bfloat16 as np_bf16
except ImportError:
    np_bf16 = None

B = 262144
D = 128
NCORES = 8
BS = B // NCORES          # rows per core
NT = BS // 128            # 128-row tiles per core (256)
XW = 130                  # x tile width: 128 features + 2 ones columns
CHUNK = 16                # tiles per input DMA (4160B per partition line)
P2B = 4                   # tiles per pass-2 matmul (512-col moving)
OBT = 16                  # tiles per output DMA (4KB per partition line)
ISQ = 1.0 / np.sqrt(128.0)


@functools.lru_cache(maxsize=1)
def _build():
    import concourse.bass as bass  # noqa: F401
    import concourse.tile as tile
    from concourse import bacc, mybir

    f32 = mybir.dt.float32
    f16 = mybir.dt.float16
    bf16 = mybir.dt.bfloat16
    AF = mybir.ActivationFunctionType
    OP = mybir.AluOpType

    nc = bacc.Bacc("TRN2", target_bir_lowering=False, debug=False,
                   num_devices=NCORES)

    x = nc.dram_tensor("x", [D, NT * XW], bf16, kind="ExternalInput").ap()
    xlo = nc.dram_tensor("xlo", [D, NT * D], bf16, kind="ExternalInput").ap()
    wot = nc.dram_tensor("wot16", [D, D], bf16, kind="ExternalInput").ap()
    consts8 = nc.dram_tensor("consts8", [D, 8, D], f32,
                             kind="ExternalInput").ap()
    consts3 = nc.dram_tensor("consts3", [D, 3], f32,
                             kind="ExternalInput").ap()
    y = nc.dram_tensor("y", [D, NT, D], f16, kind="ExternalOutput").ap()
    cc_out = nc.dram_tensor("cc", [D, 1], f32, kind="ExternalOutput").ap()

    with tile.TileContext(nc) as tc:
        with tc.tile_pool(name="const", bufs=1) as constp, \
             tc.tile_pool(name="xall", bufs=1) as xallp, \
             tc.tile_pool(name="vstore", bufs=1) as vstorep, \
             tc.tile_pool(name="small", bufs=1) as smallp, \
             tc.tile_pool(name="xin", bufs=3) as xinp, \
             tc.tile_pool(name="obp", bufs=4) as obp, \
             tc.tile_pool(name="gps", bufs=1, space="PSUM") as gps, \
             tc.tile_pool(name="vps", bufs=2, space="PSUM") as vps, \
             tc.tile_pool(name="sps", bufs=1, space="PSUM") as sps, \
             tc.tile_pool(name="p2ps", bufs=3, space="PSUM") as p2ps, \
             tc.tile_pool(name="dram", bufs=1, space="DRAM") as dramp:

            # ------------- startup: warmup + table preload + const DMAs ----
            wm = constp.tile([D, 512], bf16)
            nc.vector.memset(wm[:], 0.25)
            # preload the Exp activation table (ACT_TABLE_LOAD ~1.3us) off
            # the critical path while everything else is still DMA-ing
            ep_in = constp.tile([D, 1], f32)
            nc.scalar.memzero(ep_in[:])
            ep_out = constp.tile([D, 1], f32)
            nc.scalar.activation(ep_out[:], ep_in[:], AF.Exp)

            # constants ride the gpsimd queue so they don't delay x chunks
            wot_sb = constp.tile([D, D], bf16)
            nc.gpsimd.dma_start(wot_sb[:], wot)
            c8 = constp.tile([D, 8, D], f32)
            nc.gpsimd.dma_start(c8[:], consts8)
            c3 = constp.tile([D, 3], f32)
            nc.gpsimd.dma_start(c3[:], consts3)
            wqt_sb = c8[:, 0, :]
            wkt_sb = c8[:, 1, :]
            wv_sb = c8[:, 2, :]
            id_sb = c8[:, 3, :]
            bqrep_sb = c8[:, 4, :]
            bkrep_sb = c8[:, 5, :]
            wsrep_sb = c8[:, 6, :]
            borep_sb = c8[:, 7, :]
            bqcol_sb = c3[:, 0:1]
            bvcol_sb = c3[:, 1:3]

            # x input DMAs: 16 chunks, each 4160B contiguous per partition
            x_all = xallp.tile([D, NT * XW], bf16)
            for ch in range(NT // CHUNK):
                lo = ch * CHUNK * XW
                hi = (ch + 1) * CHUNK * XW
                nc.sync.dma_start(x_all[:, lo:hi], x[:, lo:hi])

            # ------------- G-phase: G|s accumulation over all tiles -------
            V_sb = vstorep.tile([D, NT * D], bf16)
            cinA = dramp.tile([D, D], f32)
            coutA = dramp.tile([D, D], f32)

            g_full = gps.tile([D, 512], f32, name="g")
            g_ps = g_full[:, 0:XW]
            with nc.named_scope("gphase"):
                for t in range(NT):
                    xt = x_all[:, t * XW:t * XW + 128]
                    nc.tensor.matmul(g_ps, xt, x_all[:, t * XW:(t + 1) * XW],
                                     start=(t == 0), stop=(t == NT - 1))

            # free the G accumulator bank for C: park G in SBUF
            g_sb = smallp.tile([D, XW], f32)
            nc.vector.tensor_copy(g_sb[:], g_ps)

            # G2: error-feedback residual C = xlo.T @ [xhi|1]. bf16 rounding
            # of x alone injects +-24 absolute error into the global scores,
            # and the min top-2 score gap is ~2, which flips softmax rows.
            # G + C + C.T restores ~fp32-grade scores from pure-bf16 matmuls.
            # x_lo streams through a small rotating pool (consumed once).
            c_full = gps.tile([D, 512], f32, name="c")
            c_ps = c_full[:, 0:XW]
            with nc.named_scope("g2phase"):
                for ch in range(NT // CHUNK):
                    xlc = xinp.tile([D, CHUNK * D], bf16, name="xlc")
                    nc.sync.dma_start(
                        xlc[:], xlo[:, ch * CHUNK * D:(ch + 1) * CHUNK * D])
                    for t4 in range(CHUNK):
                        t = ch * CHUNK + t4
                        xlt = xlc[:, t4 * D:(t4 + 1) * D]
                        nc.tensor.matmul(c_ps, xlt,
                                         x_all[:, t * XW:(t + 1) * XW],
                                         start=(t == 0), stop=(t == NT - 1))

            # pre-AR: scores are LINEAR in (G, s), so compute the local
            # scores contribution now and AllReduce scores (bf16) instead of
            # G -- shortens the post-AR critical chain.
            with nc.named_scope("prescore"):
                ct_ps = sps.tile([D, 512], f32, tag="sm", name="ct")
                c_sb = smallp.tile([D, XW], f32)
                nc.vector.tensor_copy(c_sb[:], c_ps)
                nc.tensor.transpose(ct_ps[:, 0:128], c_sb[:, 0:128], id_sb)
                gA_sb = smallp.tile([D, XW], f32)
                nc.vector.tensor_tensor(gA_sb[:], g_sb[:], c_sb[:], OP.add)
                nc.vector.tensor_tensor(gA_sb[:, 0:128], gA_sb[:, 0:128],
                                        ct_ps[:, 0:128], OP.add)
                # T1T = G @ WqT + s x bq
                t1_ps = sps.tile([D, 512], f32, tag="sm", name="t1")
                nc.tensor.matmul(t1_ps[:, 0:128], gA_sb[:, 0:128], wqt_sb,
                                 start=True, stop=True)
                t1_sb = smallp.tile([D, D], f32)
                nc.vector.tensor_scalar(t1_sb[:], bqrep_sb,
                                        gA_sb[:, 128:129], None, op0=OP.mult)
                nc.vector.tensor_tensor(t1_sb[:], t1_sb[:], t1_ps[:, 0:128],
                                        OP.add)
                # uT = Wq s + (B/NCORES) bq  (so the AR sum gives B bq)
                ut_ps = sps.tile([D, 512], f32, tag="sm", name="ut")
                nc.tensor.matmul(ut_ps[:, 0:2], wqt_sb, gA_sb[:, 128:130],
                                 start=True, stop=True)
                ut_sb = smallp.tile([D, 1], f32)
                nc.vector.tensor_scalar(ut_sb[:], bqcol_sb, float(B / NCORES),
                                        None, op0=OP.mult)
                nc.vector.tensor_tensor(ut_sb[:], ut_sb[:], ut_ps[:, 0:1],
                                        OP.add)
                # scores_local = T1T.T @ WkT + uT x bk   -> bf16 for the AR
                sc_ps = sps.tile([D, 512], f32, tag="sm", name="sc")
                nc.tensor.matmul(sc_ps[:, 0:128], t1_sb[:], wkt_sb,
                                 start=True, stop=True)
                scl_sb = smallp.tile([D, D], f32)
                nc.vector.tensor_scalar(scl_sb[:], bkrep_sb, ut_sb[:, :],
                                        None, op0=OP.mult)
                nc.vector.tensor_tensor(scl_sb[:], scl_sb[:],
                                        sc_ps[:, 0:128], OP.add)
                nc.gpsimd.dma_start(cinA[:], scl_sb[:])
                nc.gpsimd.collective_compute(
                    "AllReduce", OP.add,
                    replica_groups=[list(range(NCORES))],
                    ins=[cinA.opt()], outs=[coutA.opt()])
                allr = smallp.tile([D, D], f32)
                nc.sync.dma_start(allr[:], coutA[:])

            # ------------- V-phase: V_cT = X_c.T @ Wo.T (x from SBUF) -----
            with nc.named_scope("vphase"):
                for q in range(NT // 4):
                    v_ps = vps.tile([D, 4, D], f32)
                    for t4 in range(4):
                        t = q * 4 + t4
                        xt = x_all[:, t * XW:t * XW + 128]
                        nc.tensor.matmul(v_ps[:, t4, :], xt, wot_sb[:],
                                         start=True, stop=True)
                    dst = V_sb[:, q * 4 * D:(q + 1) * 4 * D]
                    if q % 2 == 0:
                        nc.scalar.activation(dst, v_ps[:], AF.Copy)
                    else:
                        nc.vector.tensor_copy(dst, v_ps[:])

            # keepalive: the PE would otherwise idle ~50us waiting for the
            # AllReduce (whose init barrier is pinned to ~70-75us after NEFF
            # start) and the HAM gate re-throttles the clock to k=4/8 after
            # ~3us of idle -- which then halves pass-2 throughput. Cheap
            # 128-col bf16 matmuls keep the clock at k=8 through the window.
            for i in range(280):
                ka_ps = p2ps.tile([D, 512], f32, tag="p2", name=f"ka{i}")
                nc.tensor.matmul(ka_ps[:, 0:128], wm[:, 0:128],
                                 wm[:, 0:128], start=True, stop=True)

            # ------------- smalls: softmax, Mv, cc ------------------------
            with nc.named_scope("smalls"):
                sc_sb = allr
                # softmax over free dim with 1/sqrt(128) scaling
                mx = smallp.tile([D, 1], f32)
                nc.vector.reduce_max(mx[:], sc_sb[:], axis=mybir.AxisListType.X)
                mxn = smallp.tile([D, 1], f32)
                nc.vector.tensor_scalar(mxn[:], mx[:], -ISQ, None, op0=OP.mult)
                wts = smallp.tile([D, D], f32)
                rs = smallp.tile([D, 1], f32)
                nc.scalar.activation(wts[:], sc_sb[:], AF.Exp,
                                     bias=mxn[:, :], scale=ISQ, accum_out=rs[:])
                ri = smallp.tile([D, 1], f32)
                nc.vector.reciprocal(ri[:], rs[:])
                nc.vector.tensor_scalar(wts[:], wts[:], ri[:, :], None,
                                        op0=OP.mult)

                # weightsT, MvT (fp16 for pass 2), cc, corr
                wt_ps = sps.tile([D, 512], f32, tag="sm", name="wt")
                nc.tensor.transpose(wt_ps[:, 0:128], wts[:], id_sb)
                wtT_sb = smallp.tile([D, D], f32)
                nc.vector.tensor_copy(wtT_sb[:], wt_ps[:, 0:128])
                mvt_ps = sps.tile([D, 512], f32, tag="sm", name="mvt")
                nc.tensor.matmul(mvt_ps[:, 0:128], wv_sb, wtT_sb[:],
                                 start=True, stop=True)
                mvt_sb = smallp.tile([D, D], bf16)
                nc.vector.tensor_copy(mvt_sb[:], mvt_ps[:, 0:128])
                cc_ps = sps.tile([D, 512], f32, tag="sm", name="cc")
                nc.tensor.matmul(cc_ps[:, 0:2], wtT_sb[:], bvcol_sb,
                                 start=True, stop=True)
                cc_sb = smallp.tile([D, 1], f32)
                nc.vector.tensor_copy(cc_sb[:], cc_ps[:, 0:1])
                # rank-1 corr term (cc x Wo@1 + bo) is added on the host
                nc.gpsimd.dma_start(cc_out[:], cc_sb[:])

            # ------------- pass 2: y_block = Mv @ V_cT (corr on host) -----
            with nc.named_scope("pass2"):
                for ob_i in range(NT // OBT):
                    ob = obp.tile([D, OBT, D], f16)
                    for j in range(OBT // P2B):
                        blk = ob_i * (OBT // P2B) + j
                        p2 = p2ps.tile([D, P2B * D], f32, tag="p2", name="p2")
                        nc.tensor.matmul(
                            p2[:], mvt_sb[:],
                            V_sb[:, blk * P2B * D:(blk + 1) * P2B * D],
                            start=True, stop=True)
                        dst = ob[:, j * P2B:(j + 1) * P2B, :]
                        src = p2[:].rearrange("p (b o) -> p b o", b=P2B)
                        if j % 2 == 0:
                            nc.vector.tensor_copy(dst, src)
                        else:
                            nc.scalar.activation(dst, src, AF.Copy)
                    if ob_i % 2 == 0:
                        nc.sync.dma_start(
                            y[:, ob_i * OBT:(ob_i + 1) * OBT, :], ob[:])
                    else:
                        nc.gpsimd.dma_start(
                            y[:, ob_i * OBT:(ob_i + 1) * OBT, :], ob[:])

    nc.compile()
    return nc


def kernel(x, Wq, bq, Wk, bk, Wv, bv, Wo, bo):
    from concourse import bass_utils

    f = np.float32
    x = np.ascontiguousarray(np.asarray(x, f))
    Wq = np.asarray(Wq, f); bq = np.asarray(bq, f)
    Wk = np.asarray(Wk, f); bk = np.asarray(bk, f)
    Wv = np.asarray(Wv, f); bv = np.asarray(bv, f)
    Wo = np.asarray(Wo, f); bo = np.asarray(bo, f)

    consts8 = np.stack([
        Wq.T, Wk.T, Wv, np.eye(D, dtype=f),
        np.broadcast_to(bq, (D, D)), np.broadcast_to(bk, (D, D)),
        np.broadcast_to(Wo.sum(1), (D, D)), np.broadcast_to(bo, (D, D)),
    ], axis=1).astype(f)
    consts3 = np.stack([bq, bv, bv], axis=1).astype(f)
    shared = {
        "wot16": np.ascontiguousarray(Wo.T.astype(np_bf16)),
        "consts8": np.ascontiguousarray(consts8),
        "consts3": np.ascontiguousarray(consts3),
    }
    # fp16 x with two ones columns, transposed per 128-row tile so each
    # partition's chunk is contiguous in DRAM: x_dev[p, t*130+d] = x[t*128+p, d]
    x_pad = np.empty((B, XW), np_bf16)
    x_pad[:, 0:128] = x
    x_pad[:, 128:XW] = 1.0
    # bf16 residual: x = x_hi + x_lo restores ~fp32-grade Gram precision
    x_lo = (x - x_pad[:, 0:128].astype(f)).astype(np_bf16)
    in_maps = []
    for s in range(NCORES):
        xs = x_pad[s * BS:(s + 1) * BS].reshape(NT, 128, XW)
        xs = np.ascontiguousarray(xs.transpose(1, 0, 2).reshape(D, NT * XW))
        xl = x_lo[s * BS:(s + 1) * BS].reshape(NT, 128, D)
        xl = np.ascontiguousarray(xl.transpose(1, 0, 2).reshape(D, NT * D))
        in_maps.append({"x": xs, "xlo": xl, **shared})

    nc = _build()
    res = bass_utils.run_bass_kernel_spmd(nc, in_maps,
                                          core_ids=list(range(NCORES)))
    kernel.last_result = res
    y16 = np.concatenate([res.results[s]["y"] for s in range(NCORES)], axis=1)
    cc = res.results[0]["cc"][:, 0].astype(f)          # [D], same on all cores
    wsum = Wo.sum(1)
    corr = cc[:, None, None] * wsum[None, None, :] + bo[None, None, :]
    y = y16.astype(f) + corr                           # [D, 8*NT, D]
    return np.ascontiguousarray(y.reshape(B, D))
